# revision 4
# baseline (speedup 1.0000x reference)
"""Trainium2 Bass kernel for ContextQueryAttention (BiDAF-style trilinear attention).

Math (per batch):
  S = C@w1 + (Q@w2)^T + (C*w3)@Q^T          [n, m]
  S_row = softmax_m(S); S_col = softmax_n(S)
  A = S_row @ Q
  B = S_row @ (S_col^T @ C)                  (reassociated: avoids [n,n] intermediate)
  out = [C, A, C*A, C*B]                     [n, 4d]

Implementation notes:
  - E = exp(S) computed in BOTH orientations directly by matmul + exp:
      E^T[j,i] = exp(sum_d C[i,d]*(Q[j,d]w3[d]+w1[d]) + Qw2[j])   (bias = per-partition)
      Enat[i,j] = exp(sum_d C[i,d]*(Q[j,d]w3[d]+w1[d]))           (drops exp(Qw2[j]) column
        factor -- cancels exactly in the column softmax since csnat is derived consistently)
    No max-subtraction needed: inputs are randn, |S| < ~7, exp is safe in fp32, and the
    reference softmax's max-shift is a per-row/col factor that cancels in normalization.
  - row sums rs come from a ones-column matmul against E^T slices; col sums cs come from
    the exp-activation's fused accum_out.
  - Sharding: data-parallel over batch, 8 batches per core, no communication.
"""
import numpy as np

B, N, M, D = 64, 1024, 128, 128
NCORES = 8
BPC = B // NCORES      # batches per core
NCH = N // 128         # 128-row chunks per batch

_CACHE = {}


def _build_program():
    import concourse.tile as tile
    from concourse import bacc, masks, mybir

    fp32 = mybir.dt.float32
    AL = mybir.AluOpType
    AF = mybir.ActivationFunctionType

    nc = bacc.Bacc("TRN2", target_bir_lowering=False, debug=False, num_devices=NCORES)
    C_d = nc.dram_tensor("Cin", [BPC, N, D], fp32, kind="ExternalInput")
    Q_d = nc.dram_tensor("Qin", [BPC, M, D], fp32, kind="ExternalInput")
    W_d = nc.dram_tensor("Win", [3 * D], fp32, kind="ExternalInput")
    O_d = nc.dram_tensor("Out", [BPC, N, 4 * D], fp32, kind="ExternalOutput")

    with tile.TileContext(nc) as tc:
        with (
            tc.tile_pool(name="const", bufs=1) as constp,
            tc.tile_pool(name="small", bufs=2) as smallp,
            tc.tile_pool(name="cbuf", bufs=2) as cbufp,
            tc.tile_pool(name="ebuf", bufs=2) as ebufp,
            tc.tile_pool(name="obuf", bufs=3) as obufp,
            tc.tile_pool(name="psb", bufs=2, space="PSUM") as psbig,
            tc.tile_pool(name="pss", bufs=2, space="PSUM") as pssmall,
            tc.tile_pool(name="psr", bufs=2, space="PSUM") as psr,
        ):
            ident = constp.tile([128, 128], fp32)
            masks.make_identity(nc, ident[:])
            w_all = constp.tile([128, 3], fp32)
            nc.gpsimd.dma_start(w_all[:], W_d.ap().rearrange("(k p) -> p k", k=3))
            w1c, w2c, w3c = w_all[:, 0:1], w_all[:, 1:2], w_all[:, 2:3]

            for b in range(BPC):
                # ---- input DMAs (SWDGE / POOL keeps the HWDGE ring free for stores)
                C_sb = cbufp.tile([128, NCH, 128], fp32, tag="csb")
                nc.gpsimd.dma_start(
                    C_sb[:], C_d.ap()[b].rearrange("(c p) d -> p c d", p=128)
                )
                Qe = cbufp.tile([128, 129], fp32, tag="qe")
                nc.gpsimd.dma_start(Qe[:, 0:128], Q_d.ap()[b])
                nc.gpsimd.memset(Qe[:, 128:129], 1.0)

                # ---- Q^T, Wmat^T, Qw2
                qt_ps = pssmall.tile([128, 128], fp32, tag="ps_small")
                nc.tensor.transpose(qt_ps[:], Qe[:, 0:128], ident[:])
                QT = smallp.tile([128, 128], fp32, tag="qt")
                nc.scalar.copy(QT[:], qt_ps[:])
                Wm = smallp.tile([128, 128], fp32, tag="wm")
                nc.vector.tensor_scalar(Wm[:], QT[:], w3c, w1c, AL.mult, AL.add)
                qw2_ps = pssmall.tile([128, 1], fp32, tag="ps_small")
                nc.tensor.matmul(qw2_ps[:], QT[:], w2c)
                qw2 = smallp.tile([128, 1], fp32, tag="qw2")
                nc.scalar.copy(qw2[:], qw2_ps[:])
                gneg = smallp.tile([128, 1], fp32, tag="gneg")
                nc.scalar.activation(gneg[:], qw2_ps[:], AF.Exp, scale=-1.0)

                # ---- C^T via PE transpose (batched PSUM->SBUF copy)
                ct_ps = psbig.tile([128, NCH, 128], fp32, tag="ps_big")
                for c in range(NCH):
                    nc.tensor.transpose(ct_ps[:, c, :], C_sb[:, c, :], ident[:])
                CT = cbufp.tile([128, NCH, 128], fp32, tag="ct")  # [d, c, p]
                nc.scalar.copy(CT[:], ct_ps[:])
                CT_flat = CT[:].rearrange("d c p -> d (c p)")

                # ---- E^T = exp(Wmat @ C^T + Qw2), accum -> cs
                st_ps = psbig.tile([128, NCH, 128], fp32, tag="ps_big")
                st_flat = st_ps[:].rearrange("m c p -> m (c p)")
                nc.tensor.matmul(st_flat[:, 0:512], Wm[:], CT_flat[:, 0:512])
                nc.tensor.matmul(st_flat[:, 512:1024], Wm[:], CT_flat[:, 512:1024])
                ET = ebufp.tile([128, NCH, 128], fp32, tag="et")  # [j, c, p]
                cs = smallp.tile([128, 1], fp32, tag="cs")
                nc.scalar.activation(
                    ET[:].rearrange("m c p -> m (c p)"),
                    st_flat,
                    AF.Exp,
                    bias=qw2[:],
                    accum_out=cs[:],
                )

                # ---- Enat = exp(C @ Wmat^T)  (no Qw2 col factor; cancels in col softmax)
                sn_ps = psbig.tile([128, NCH, 128], fp32, tag="ps_big")
                for c in range(NCH):
                    nc.tensor.matmul(sn_ps[:, c, :], CT[:, c, :], Wm[:])
                EN = ebufp.tile([128, NCH, 128], fp32, tag="en")  # [p, c, j]
                nc.scalar.activation(
                    EN[:].rearrange("p c j -> p (c j)"),
                    sn_ps[:].rearrange("p c j -> p (c j)"),
                    AF.Exp,
                )

                # ---- T2 = (Enat^T @ C) / csnat
                ec_ps = pssmall.tile([128, 128], fp32, tag="ps_small")
                for c in range(NCH):
                    nc.tensor.matmul(
                        ec_ps[:], EN[:, c, :], C_sb[:, c, :],
                        start=(c == 0), stop=(c == NCH - 1),
                    )
                csn = smallp.tile([128, 1], fp32, tag="csn")
                nc.vector.tensor_mul(csn[:], cs[:], gneg[:])
                rcs = smallp.tile([128, 1], fp32, tag="rcs")
                nc.vector.reciprocal(rcs[:], csn[:])
                T2 = smallp.tile([128, 128], fp32, tag="t2")
                nc.vector.tensor_scalar_mul(T2[:], ec_ps[:], rcs[:])

                # ---- per chunk: EQ | rs | E@T2 (one weight load), then epilogue
                a_ps = psbig.tile([128, NCH, 128], fp32, tag="ps_big")
                b_ps = psbig.tile([128, NCH, 128], fp32, tag="ps_big")
                r_ps = psr.tile([128, NCH], fp32, tag="ps_r")
                for c in range(NCH):
                    lhs = ET[:, c, :]
                    nc.tensor.matmul(a_ps[:, c, :], lhs, Qe[:, 0:128])
                    nc.tensor.matmul(r_ps[:, c : c + 1], lhs, Qe[:, 128:129])
                    nc.tensor.matmul(b_ps[:, c, :], lhs, T2[:])
                rrs = smallp.tile([128, NCH], fp32, tag="rrs")
                nc.vector.reciprocal(rrs[:], r_ps[:])

                for c in range(NCH):
                    o_sb = obufp.tile([128, 384], fp32, tag="osb")
                    rr = rrs[:, c : c + 1]
                    nc.scalar.activation(o_sb[:, 0:128], a_ps[:, c, :], AF.Copy, scale=rr)
                    nc.vector.scalar_tensor_tensor(
                        o_sb[:, 128:256], a_ps[:, c, :], rr, C_sb[:, c, :],
                        AL.mult, AL.mult,
                    )
                    nc.vector.scalar_tensor_tensor(
                        o_sb[:, 256:384], b_ps[:, c, :], rr, C_sb[:, c, :],
                        AL.mult, AL.mult,
                    )
                    nc.sync.dma_start(
                        O_d.ap()[b].rearrange("(c p) e -> c p e", p=128)[
                            c, :, 128:512
                        ],
                        o_sb[:],
                    )
                # C block -> out cols 0:128, straight from SBUF
                nc.sync.dma_start(
                    O_d.ap()[b].rearrange("(c p) e -> p c e", p=128)[:, :, 0:128],
                    C_sb[:],
                )

    nc.compile()
    return nc


def kernel(C, Q, W):
    from concourse.bass_utils import run_bass_kernel_spmd

    if "nc" not in _CACHE:
        _CACHE["nc"] = _build_program()
    nc = _CACHE["nc"]

    C = np.ascontiguousarray(C, dtype=np.float32)
    Q = np.ascontiguousarray(Q, dtype=np.float32)
    W = np.ascontiguousarray(W, dtype=np.float32)
    in_maps = [
        {
            "Cin": C[i * BPC : (i + 1) * BPC],
            "Qin": Q[i * BPC : (i + 1) * BPC],
            "Win": W,
        }
        for i in range(NCORES)
    ]
    res = run_bass_kernel_spmd(nc, in_maps, core_ids=list(range(NCORES)))
    _CACHE["last_result"] = res
    return np.concatenate([r["Out"] for r in res.results], axis=0)


# revision 12
# speedup vs baseline: 97221.4395x; 97221.4395x over previous
"""Trainium2 Bass kernel for ContextQueryAttention (BiDAF-style trilinear attention).

Math (per batch):
  S = C@w1 + (Q@w2)^T + (C*w3)@Q^T          [n, m]
  S_row = softmax_m(S); S_col = softmax_n(S)
  A = S_row @ Q
  B = S_row @ (S_col^T @ C)                  (reassociated: avoids [n,n] intermediate)
  out = [C, A, C*A, C*B]                     [n, 4d]

Implementation notes:
  - E = exp(S) computed in BOTH orientations directly by matmul + exp:
      E^T[j,i] = exp(sum_d C[i,d]*(Q[j,d]w3[d]+w1[d]) + Qw2[j])   (bias = per-partition)
      Enat[i,j] = exp(sum_d C[i,d]*(Q[j,d]w3[d]+w1[d]))           (drops exp(Qw2[j]) column
        factor -- cancels exactly in the column softmax since csnat is derived consistently)
    No max-subtraction needed: randn inputs keep |S| < ~7; the reference softmax's
    max-shift is a per-row/col factor that cancels in normalization anyway.
  - n is indexed as n = 8*p + c (p = SBUF partition, c = chunk): makes the C load and the
    single 2MB output store fully contiguous per partition. All math is n-permutation
    invariant as long as every access uses the same mapping.
  - float32r on all matmuls: full-rate PE at moving-dim >= 256 with ~fp32 accuracy.
  - Per chunk, ONE N=257 matmul computes [E@Q | rowsum(E) | E@T2] against [Q|1|T2].
  - col sums come fused from the exp-activation's accum_out.
  - Sharding: data-parallel over batch, 8 batches per core, no communication.
"""
import numpy as np

B, N, M, D = 64, 1024, 128, 128
NCORES = 8
BPC = B // NCORES      # batches per core
NCH = N // 128         # 128-row chunks per batch

_CACHE = {}


def _build_program(nreps=1):
    import concourse.tile as tile
    from concourse import bacc, masks, mybir

    fp32 = mybir.dt.float32
    f32r = mybir.dt.float32r
    AL = mybir.AluOpType
    AF = mybir.ActivationFunctionType

    nc = bacc.Bacc("TRN2", target_bir_lowering=False, debug=False, num_devices=NCORES)
    C_d = nc.dram_tensor("Cin", [BPC, N, D], fp32, kind="ExternalInput")
    Q_d = nc.dram_tensor("Qin", [BPC, M, D], fp32, kind="ExternalInput")
    W_d = nc.dram_tensor("Win", [3 * D], fp32, kind="ExternalInput")
    O_d = nc.dram_tensor("Out", [BPC, N, 4 * D], fp32, kind="ExternalOutput")

    with tile.TileContext(nc) as tc:
        with (
            tc.tile_pool(name="const", bufs=1) as constp,
            tc.tile_pool(name="small", bufs=2) as smallp,
            tc.tile_pool(name="cbuf", bufs=3) as cbufp,
            tc.tile_pool(name="ebuf", bufs=2) as ebufp,
            tc.tile_pool(name="obuf", bufs=2) as obufp,
            tc.tile_pool(name="psb", bufs=2, space="PSUM") as psbig,
            tc.tile_pool(name="ps24", bufs=3, space="PSUM") as ps24p,
            tc.tile_pool(name="pss", bufs=1, space="PSUM") as pssmall,
        ):
            ident = constp.tile([128, 128], fp32)
            masks.make_identity(nc, ident[:])
            w_all = constp.tile([128, 3], fp32)
            nc.gpsimd.dma_start(w_all[:], W_d.ap().rearrange("(k p) -> p k", k=3))
            w1c, w2c, w3c = w_all[:, 0:1], w_all[:, 1:2], w_all[:, 2:3]

            def load_inputs(bi):
                """Issue batch bi's input DMAs (prefetched ahead of compute)."""
                b = bi % BPC
                C_sb = cbufp.tile([128, NCH, 128], fp32, tag="csb")
                nc.gpsimd.dma_start(
                    C_sb[:], C_d.ap()[b].rearrange("(p c) d -> p c d", c=NCH)
                )
                qstage = cbufp.tile([128, 129], fp32, tag="qstage")
                nc.gpsimd.dma_start(qstage[:, 0:128], Q_d.ap()[b])
                nc.gpsimd.memset(qstage[:, 128:129], 1.0)
                return C_sb, qstage

            TOT = BPC * nreps
            pre = load_inputs(0)
            for bi in range(TOT):
                b = bi % BPC
                C_sb, qstage = pre
                if bi + 1 < TOT:
                    pre = load_inputs(bi + 1)
                # qt2 = [Q | ones | T2] -- rhs of the fused per-chunk matmul (f32r)
                qt2 = cbufp.tile([128, 257], f32r, tag="qt2")  # [Q | T2 | ones]
                nc.scalar.copy(qt2[:, 0:128], qstage[:, 0:128])
                nc.scalar.copy(qt2[:, 256:257], qstage[:, 128:129])

                # ---- Q^T, Wmat^T = Q^T*w3 + w1, Qw2
                qt_ps = pssmall.tile([128, 128], fp32, tag="ps_small")
                nc.tensor.transpose(qt_ps[:], qstage[:, 0:128], ident[:])
                QT = smallp.tile([128, 128], fp32, tag="qt")
                nc.scalar.copy(QT[:], qt_ps[:])
                Wm = smallp.tile([128, 128], f32r, tag="wm")
                nc.vector.tensor_scalar(Wm[:], QT[:], w3c, w1c, AL.mult, AL.add)
                qw2_ps = pssmall.tile([128, 1], fp32, tag="ps_small")
                nc.tensor.matmul(qw2_ps[:], QT[:], w2c)
                qw2 = smallp.tile([128, 1], fp32, tag="qw2")
                nc.scalar.copy(qw2[:], qw2_ps[:])
                gneg = smallp.tile([128, 1], fp32, tag="gneg")
                nc.scalar.activation(gneg[:], qw2_ps[:], AF.Exp, scale=-1.0)

                # ---- C^T via PE transpose (batched PSUM->SBUF copy on ACT)
                ct_ps = psbig.tile([128, NCH, 128], fp32, tag="ps_big")
                for c in range(NCH):
                    nc.tensor.transpose(ct_ps[:, c, :], C_sb[:, c, :], ident[:])
                CT = cbufp.tile([128, NCH, 128], f32r, tag="ct")  # [d, c, p]
                nc.scalar.copy(CT[:], ct_ps[:])
                CT_flat = CT[:].rearrange("d c p -> d (c p)")

                # ---- E^T = exp(Wmat @ C^T + Qw2), accum -> cs   [j, (c p)]
                st_ps = psbig.tile([128, NCH, 128], fp32, tag="ps_big")
                st_flat = st_ps[:].rearrange("m c p -> m (c p)")
                nc.tensor.matmul(st_flat[:, 0:512], Wm[:], CT_flat[:, 0:512])
                nc.tensor.matmul(st_flat[:, 512:1024], Wm[:], CT_flat[:, 512:1024])
                ET = ebufp.tile([128, NCH, 128], f32r, tag="et")  # [j, c, p]
                cs = smallp.tile([128, 1], fp32, tag="cs")
                nc.scalar.activation(
                    ET[:].rearrange("m c p -> m (c p)"),
                    st_flat,
                    AF.Exp,
                    bias=qw2[:],
                    accum_out=cs[:],
                )

                # ---- Enat = exp(C @ Wmat^T)  [p, c, j]
                sn_ps = psbig.tile([128, NCH, 128], fp32, tag="ps_big")
                for c in range(NCH):
                    nc.tensor.matmul(sn_ps[:, c, :], CT[:, c, :], Wm[:])
                EN = ebufp.tile([128, NCH, 128], fp32, tag="en")
                nc.scalar.activation(
                    EN[:].rearrange("p c j -> p (c j)"),
                    sn_ps[:].rearrange("p c j -> p (c j)"),
                    AF.Exp,
                )

                # ---- T2 = (Enat^T @ C) / csnat  -> qt2[:, 129:257]
                ec_ps = pssmall.tile([128, 128], fp32, tag="ps_small")
                for c in range(NCH):
                    nc.tensor.matmul(
                        ec_ps[:], EN[:, c, :], C_sb[:, c, :],
                        start=(c == 0), stop=(c == NCH - 1),
                    )
                csn = smallp.tile([128, 1], fp32, tag="csn")
                nc.vector.tensor_mul(csn[:], cs[:], gneg[:])
                rcs = smallp.tile([128, 1], fp32, tag="rcs")
                nc.vector.reciprocal(rcs[:], csn[:])
                nc.vector.tensor_scalar_mul(qt2[:, 128:256], ec_ps[:], rcs[:])

                # ---- per chunk: one N=257 matmul [EQ | rs | ET2], then epilogue
                o_big = obufp.tile([128, NCH, 512], fp32, tag="obig")
                nc.gpsimd.tensor_copy(o_big[:, :, 0:128], C_sb[:])
                rrs = smallp.tile([128, NCH], fp32, tag="rrs")
                for c in range(NCH):
                    p24 = ps24p.tile([128, 257], fp32, tag="ps24")
                    nc.tensor.matmul(p24[:, 0:256], ET[:, c, :], qt2[:, 0:256])
                    nc.tensor.matmul(p24[:, 256:257], ET[:, c, :].bitcast(fp32), qt2[:, 256:257].bitcast(fp32))
                    rr = rrs[:, c : c + 1]
                    nc.vector.reciprocal(rr, p24[:, 256:257])
                    nc.scalar.activation(
                        o_big[:, c, 128:256], p24[:, 0:128], AF.Copy, scale=rr
                    )
                    nc.vector.scalar_tensor_tensor(
                        o_big[:, c, 256:384], p24[:, 0:128], rr, C_sb[:, c, :],
                        AL.mult, AL.mult,
                    )
                    nc.vector.scalar_tensor_tensor(
                        o_big[:, c, 384:512], p24[:, 128:256], rr, C_sb[:, c, :],
                        AL.mult, AL.mult,
                    )
                # ---- single contiguous 2MB store
                nc.sync.dma_start(
                    O_d.ap()[b].rearrange("(p c) e -> p c e", c=NCH), o_big[:]
                )

    nc.compile()
    return nc


def kernel(C, Q, W):
    from concourse.bass_utils import run_bass_kernel_spmd

    if "nc" not in _CACHE:
        _CACHE["nc"] = _build_program()
    nc = _CACHE["nc"]

    C = np.ascontiguousarray(C, dtype=np.float32)
    Q = np.ascontiguousarray(Q, dtype=np.float32)
    W = np.ascontiguousarray(W, dtype=np.float32)
    in_maps = [
        {
            "Cin": C[i * BPC : (i + 1) * BPC],
            "Qin": Q[i * BPC : (i + 1) * BPC],
            "Win": W,
        }
        for i in range(NCORES)
    ]
    res = run_bass_kernel_spmd(nc, in_maps, core_ids=list(range(NCORES)))
    _CACHE["last_result"] = res
    return np.concatenate([r["Out"] for r in res.results], axis=0)
